# revision 7
# baseline (speedup 1.0000x reference)
"""Trainium2 Bass kernel for sparse-conv (kernel-map gather-GEMM-scatter).

Math: out[j, d] = sum over points i with out_idx[i]==j of  x[i, :] @ W[k_idx[i], :, d]

Device strategy ("lane-plane compact layout"):
  Each output voxel j has m_j occupied kernel offsets (avg ~3.1 of 8). Instead
  of a dense 8-slot expansion (62% zeros), store only occupied slots:

  - Points sorted by (voxel, k); voxels sorted globally by (m desc, k-bitmask)
    and dealt round-robin onto 32 lanes = 8 cores x 4 PE quadrant-lanes, so
    all cores share ONE static schedule (SPMD) with tiny padding.
  - Per lane a voxel is one PSUM column; its m points live in m "planes".
    Plane q holds each voxel's q-th point; since columns are m-desc sorted,
    plane q covers a prefix of the columns.
  - A matmul per (chunk of 512 cols, plane, k-run) contracts K=32 channels
    with W[k] using tile_position=(32a,32a): the 4 lanes of a core run
    concurrently in 4 PE quadrants, accumulating into disjoint 32-partition
    slices of one PSUM bank.
  - PSUM -> bf16 staging -> HBM. Host decodes voxel order.

  Per-core HBM traffic: ~6.5 MB in + 2.1 MB out (bf16) vs 21 MB for the dense
  bf16 layout — the kernel is DMA-bound, so bytes are the metric to minimize.
"""
import sys

if "/opt/trn_rl_repo" not in sys.path:
    sys.path.insert(0, "/opt/trn_rl_repo")

import numpy as np

N_CORES = 8
LANES = 32          # 8 cores x 4 PE lanes
CHUNK = 512         # voxel columns per PSUM bank
STAGE = 2           # chunks per output staging block
DT_IN = "bfloat16"
DMA_PAIR = 4        # chunks per input DMA

_prog_cache = {}


# ----------------------------------------------------------------- packing --
def _pack(x, W, k_idx, out_idx, num_out):
    """Returns (in_maps, schedule, decode_info)."""
    import ml_dtypes
    bf16 = ml_dtypes.bfloat16

    n = x.shape[0]
    out_idx = out_idx.astype(np.int64)
    k_idx = k_idx.astype(np.int64)

    order = np.lexsort((k_idx, out_idx))          # points by (voxel, k)
    vox_pt = out_idx[order]
    k_pt = k_idx[order]

    m = np.bincount(out_idx, minlength=num_out)
    starts = np.zeros(num_out + 1, np.int64)
    np.cumsum(m, out=starts[1:])
    q_pt = np.arange(n, dtype=np.int64) - starts[vox_pt]
    dup = m.max() > 8                              # duplicate (voxel,k) pairs?

    bm = np.bitwise_or.reduceat(1 << k_pt, starts[:-1])

    gkey = (8 - np.minimum(m, 8)) * 256 + bm       # voxel sort: m desc, bm asc
    vorder = np.argsort(gkey, kind="stable")
    gkey_s = gkey[vorder]

    uniq, gcounts = np.unique(gkey_s, return_counts=True)
    n_groups = len(uniq)
    g_m = 8 - (uniq >> 8)
    g_bm = uniq & 255
    w_g = -(-gcounts // LANES)
    colbase = np.zeros(n_groups + 1, np.int64)
    np.cumsum(w_g, out=colbase[1:])
    C = int(colbase[-1])
    C_pad = -(-C // (CHUNK * STAGE)) * (CHUNK * STAGE)
    T = C_pad // CHUNK
    NBLK = T // STAGE

    gstart_s = np.zeros(n_groups + 1, np.int64)
    np.cumsum(gcounts, out=gstart_s[1:])
    svi = np.arange(num_out, dtype=np.int64)
    ginv = np.searchsorted(gstart_s, svi, side="right") - 1
    r = svi - gstart_s[ginv]
    v_lane = r % LANES
    v_col = colbase[ginv] + r // LANES

    # per-column metadata (lane-independent); pad C..C_pad with virtual m=1,k=0
    gcol = np.repeat(np.arange(n_groups), w_g)
    m_col = np.concatenate([g_m[gcol], np.ones(C_pad - C, np.int64)])
    bm_col = np.concatenate([g_bm[gcol], np.ones(C_pad - C, np.int64)])

    M_q = np.array([(m_col > q).sum() for q in range(8)], dtype=np.int64)
    P = np.zeros((T, 8), np.int64)
    for t in range(T):
        P[t] = np.clip(M_q - CHUNK * t, 0, CHUNK)
    Poff = np.zeros((T, 8), np.int64)
    np.cumsum(P[:, :7], axis=1, out=Poff[:, 1:])
    blk_w = P.sum(axis=1)
    B = np.zeros(T + 1, np.int64)
    np.cumsum(blk_w, out=B[1:])
    CTOT = int(B[-1])

    bit_tab = np.full((256, 8), 0, np.int64)
    for b in range(256):
        ks = [k for k in range(8) if b >> k & 1]
        bit_tab[b, :len(ks)] = ks

    vrank = np.empty(num_out, np.int64)
    vrank[vorder] = svi
    pv = vrank[vox_pt]
    p_lane = v_lane[pv]
    p_vcol = v_col[pv]
    p_core = p_lane // 4
    p_a = p_lane % 4
    p_t = p_vcol // CHUNK
    p_col = B[p_t] + Poff[p_t, q_pt] + p_vcol % CHUNK

    xin = np.zeros((N_CORES, 128, CTOT), np.float32)
    rows = (32 * p_a)[:, None] + np.arange(32)[None, :]
    if dup:
        np.add.at(xin, (p_core[:, None], rows, p_col[:, None]), x[order])
    else:
        xin[p_core[:, None], rows, p_col[:, None]] = x[order]

    runs = []
    for t in range(T):
        rl = []
        for q in range(8):
            pq = int(P[t, q])
            if pq == 0:
                break
            seg_m = m_col[CHUNK * t: CHUNK * t + pq]
            seg_bm = bm_col[CHUNK * t: CHUNK * t + pq]
            seg_k = bit_tab[seg_bm, q]
            stop_f = (seg_m == q + 1)
            key = seg_k * 2 + stop_f
            bnd = np.flatnonzero(np.diff(key)) + 1
            bnds = np.concatenate(([0], bnd, [pq]))
            for c0, c1 in zip(bnds[:-1], bnds[1:]):
                rl.append((q, int(c0), int(c1), int(seg_k[c0]),
                           q == 0, bool(stop_f[c0])))
        runs.append(rl)

    # wdiag: col block k (128 wide) holds W[k] replicated on the 4 diagonal
    # 32x32 blocks -> one K=128 matmul computes all 4 lanes at once.
    # Block 8 is zeros for the full-bank start=True PSUM zero-init.
    wrep = np.zeros((128, 9 * 128), np.float32)
    for k in range(8):
        for a in range(4):
            wrep[32 * a:32 * a + 32, 128 * k + 32 * a:128 * k + 32 * a + 32] = W[k]

    in_maps = [{"xin": xin[c].astype(bf16), "wrep": wrep.astype(bf16)}
               for c in range(N_CORES)]

    sched = (CTOT, T, NBLK, tuple(B.tolist()),
             tuple(tuple(row) for row in Poff.tolist()),
             tuple(tuple(rl) for rl in runs))
    dec = dict(vorder=vorder, d_core=v_lane // 4, d_a=v_lane % 4,
               d_blk=v_col // (CHUNK * STAGE),
               d_col=(v_col // CHUNK % STAGE) * CHUNK + v_col % CHUNK,
               num_out=num_out)
    return in_maps, sched, dec


# ----------------------------------------------------------------- program --
def _build_program(sched, dt_name):
    import concourse.tile as tile
    from concourse import bacc, mybir

    CTOT, T, NBLK, B, Poff, runs = sched
    dt = getattr(mybir.dt, dt_name)
    f32 = mybir.dt.float32

    nc = bacc.Bacc("TRN2", target_bir_lowering=False, debug=False)
    xin_d = nc.dram_tensor("xin", [128, CTOT], dt, kind="ExternalInput")
    w_d = nc.dram_tensor("wrep", [128, 9 * 128], dt, kind="ExternalInput")
    out_d = nc.dram_tensor("out_st", [NBLK, 128, CHUNK * STAGE], dt,
                           kind="ExternalOutput")

    with tile.TileContext(nc) as tc:
        with (
            tc.tile_pool(name="w", bufs=1) as wpool,
            tc.tile_pool(name="xb", bufs=1) as xpool,
            tc.tile_pool(name="st", bufs=2) as stpool,
            tc.tile_pool(name="ps", bufs=8, space="PSUM") as pspool,
        ):
            wt = wpool.tile([128, 9 * 128], dt, tag="wt")
            nc.gpsimd.dma_start(wt[:], w_d.ap()[:, :])

            xin = xpool.tile([128, CTOT], dt, tag="xin")
            for t0 in range(0, T, DMA_PAIR):
                c0, c1 = B[t0], B[min(t0 + DMA_PAIR, T)]
                eng = nc.sync if (t0 // DMA_PAIR) % 2 == 0 else nc.scalar
                eng.dma_start(xin[:, c0:c1], xin_d.ap()[:, c0:c1])

            for blk in range(NBLK):
                staging = stpool.tile([128, CHUNK * STAGE], dt, tag="stg")
                for s in range(STAGE):
                    t = blk * STAGE + s
                    ps = pspool.tile([128, CHUNK], f32, tag="ps")
                    # zero-init whole bank + set has_written in ONE instruction
                    # so every data matmul is a pure commutative accumulate
                    nc.tensor.matmul(
                        ps[:, :], wt[:, 8 * 128:9 * 128],
                        xin[:, B[t]:B[t] + CHUNK],
                        start=True, stop=False, skip_group_check=True)
                    # k-sorted so consecutive matmuls share stationary weights
                    for (q, c0, c1, k, start, stop) in sorted(
                            runs[t], key=lambda r: (r[3], r[0], r[1])):
                        off = B[t] + Poff[t][q]
                        nc.tensor.matmul(
                            ps[:, c0:c1],
                            wt[:, 128 * k:128 * k + 128],
                            xin[:, off + c0:off + c1],
                            start=False, stop=stop, skip_group_check=True)
                    dst = staging[:, CHUNK * s:CHUNK * (s + 1)]
                    if s % 2 == 0:
                        nc.vector.tensor_copy(dst, ps[:])
                    else:
                        nc.scalar.copy(dst, ps[:])
                nc.gpsimd.dma_start(out_d.ap()[blk], staging[:])

    nc.compile()
    return nc


def _get_program(sched, dt_name):
    key = (hash(sched), dt_name)
    if key not in _prog_cache:
        _prog_cache[key] = _build_program(sched, dt_name)
    return _prog_cache[key]


def _decode(results, dec):
    num_out = dec["num_out"]
    out_st = np.stack([r["out_st"].astype(np.float32) for r in results])
    rows = (32 * dec["d_a"])[:, None] + np.arange(32)[None, :]
    vals = out_st[dec["d_core"][:, None], dec["d_blk"][:, None], rows,
                  dec["d_col"][:, None]]
    res = np.empty((num_out, 32), np.float32)
    res[dec["vorder"]] = vals
    return res


def run(x, W, k_idx, out_idx, num_out, trace=False, dt_name=DT_IN):
    from concourse.bass_utils import run_bass_kernel_spmd

    x = np.asarray(x, dtype=np.float32)
    W = np.asarray(W, dtype=np.float32)
    k_idx = np.asarray(k_idx, dtype=np.int32)
    out_idx = np.asarray(out_idx, dtype=np.int32)
    num_out = int(num_out)

    in_maps, sched, dec = _pack(x, W, k_idx, out_idx, num_out)
    nc = _get_program(sched, dt_name)
    res = run_bass_kernel_spmd(nc, in_maps, list(range(N_CORES)), trace=trace)
    out = _decode(res.results, dec)
    return out, res


def kernel(x, W, k_idx, out_idx, num_out):
    out, _ = run(x, W, k_idx, out_idx, num_out, trace=False)
    return out


# revision 8
# speedup vs baseline: 1.1100x; 1.1100x over previous
"""Trainium2 Bass kernel for sparse-conv (kernel-map gather-GEMM-scatter).

Math: out[j, d] = sum over points i with out_idx[i]==j of  x[i, :] @ W[k_idx[i], :, d]

Device strategy ("lane-plane compact layout"):
  Each output voxel j has m_j occupied kernel offsets (avg ~3.1 of 8). Instead
  of a dense 8-slot expansion (62% zeros), store only occupied slots:

  - Points sorted by (voxel, k); voxels sorted globally by (m desc, k-bitmask)
    and dealt round-robin onto 32 lanes = 8 cores x 4 PE quadrant-lanes, so
    all cores share ONE static schedule (SPMD) with tiny padding.
  - Per lane a voxel is one PSUM column; its m points live in m "planes".
    Plane q holds each voxel's q-th point; since columns are m-desc sorted,
    plane q covers a prefix of the columns.
  - A matmul per (chunk of 512 cols, plane, k-run) contracts K=32 channels
    with W[k] using tile_position=(32a,32a): the 4 lanes of a core run
    concurrently in 4 PE quadrants, accumulating into disjoint 32-partition
    slices of one PSUM bank.
  - PSUM -> bf16 staging -> HBM. Host decodes voxel order.

  Per-core HBM traffic: ~6.5 MB in + 2.1 MB out (bf16) vs 21 MB for the dense
  bf16 layout — the kernel is DMA-bound, so bytes are the metric to minimize.
"""
import sys

if "/opt/trn_rl_repo" not in sys.path:
    sys.path.insert(0, "/opt/trn_rl_repo")

import numpy as np

N_CORES = 8
LANES = 32          # 8 cores x 4 PE lanes
CHUNK = 512         # voxel columns per PSUM bank
STAGE = 4           # chunks per output staging block
DT_IN = "bfloat16"
DMA_PAIR = 2        # chunks per input DMA

_prog_cache = {}


# ----------------------------------------------------------------- packing --
def _pack(x, W, k_idx, out_idx, num_out):
    """Returns (in_maps, schedule, decode_info)."""
    import ml_dtypes
    bf16 = ml_dtypes.bfloat16

    n = x.shape[0]
    out_idx = out_idx.astype(np.int64)
    k_idx = k_idx.astype(np.int64)

    order = np.lexsort((k_idx, out_idx))          # points by (voxel, k)
    vox_pt = out_idx[order]
    k_pt = k_idx[order]

    m = np.bincount(out_idx, minlength=num_out)
    starts = np.zeros(num_out + 1, np.int64)
    np.cumsum(m, out=starts[1:])
    q_pt = np.arange(n, dtype=np.int64) - starts[vox_pt]
    dup = m.max() > 8                              # duplicate (voxel,k) pairs?

    bm = np.bitwise_or.reduceat(1 << k_pt, starts[:-1])

    gkey = (8 - np.minimum(m, 8)) * 256 + bm       # voxel sort: m desc, bm asc
    vorder = np.argsort(gkey, kind="stable")
    gkey_s = gkey[vorder]

    uniq, gcounts = np.unique(gkey_s, return_counts=True)
    n_groups = len(uniq)
    g_m = 8 - (uniq >> 8)
    g_bm = uniq & 255
    w_g = -(-gcounts // LANES)
    colbase = np.zeros(n_groups + 1, np.int64)
    np.cumsum(w_g, out=colbase[1:])
    C = int(colbase[-1])
    C_pad = -(-C // (CHUNK * STAGE)) * (CHUNK * STAGE)
    T = C_pad // CHUNK
    NBLK = T // STAGE

    gstart_s = np.zeros(n_groups + 1, np.int64)
    np.cumsum(gcounts, out=gstart_s[1:])
    svi = np.arange(num_out, dtype=np.int64)
    ginv = np.searchsorted(gstart_s, svi, side="right") - 1
    r = svi - gstart_s[ginv]
    v_lane = r % LANES
    v_col = colbase[ginv] + r // LANES

    # per-column metadata (lane-independent); pad C..C_pad with virtual m=1,k=0
    gcol = np.repeat(np.arange(n_groups), w_g)
    m_col = np.concatenate([g_m[gcol], np.ones(C_pad - C, np.int64)])
    bm_col = np.concatenate([g_bm[gcol], np.ones(C_pad - C, np.int64)])

    M_q = np.array([(m_col > q).sum() for q in range(8)], dtype=np.int64)
    P = np.zeros((T, 8), np.int64)
    for t in range(T):
        P[t] = np.clip(M_q - CHUNK * t, 0, CHUNK)
    Poff = np.zeros((T, 8), np.int64)
    np.cumsum(P[:, :7], axis=1, out=Poff[:, 1:])
    blk_w = P.sum(axis=1)
    B = np.zeros(T + 1, np.int64)
    np.cumsum(blk_w, out=B[1:])
    CTOT = int(B[-1])

    bit_tab = np.full((256, 8), 0, np.int64)
    for b in range(256):
        ks = [k for k in range(8) if b >> k & 1]
        bit_tab[b, :len(ks)] = ks

    vrank = np.empty(num_out, np.int64)
    vrank[vorder] = svi
    pv = vrank[vox_pt]
    p_lane = v_lane[pv]
    p_vcol = v_col[pv]
    p_core = p_lane // 4
    p_a = p_lane % 4
    p_t = p_vcol // CHUNK
    p_col = B[p_t] + Poff[p_t, q_pt] + p_vcol % CHUNK

    xin = np.zeros((N_CORES, 128, CTOT), np.float32)
    rows = (32 * p_a)[:, None] + np.arange(32)[None, :]
    if dup:
        np.add.at(xin, (p_core[:, None], rows, p_col[:, None]), x[order])
    else:
        xin[p_core[:, None], rows, p_col[:, None]] = x[order]

    runs = []
    for t in range(T):
        rl = []
        for q in range(8):
            pq = int(P[t, q])
            if pq == 0:
                break
            seg_m = m_col[CHUNK * t: CHUNK * t + pq]
            seg_bm = bm_col[CHUNK * t: CHUNK * t + pq]
            seg_k = bit_tab[seg_bm, q]
            stop_f = (seg_m == q + 1)
            key = seg_k * 2 + stop_f
            bnd = np.flatnonzero(np.diff(key)) + 1
            bnds = np.concatenate(([0], bnd, [pq]))
            for c0, c1 in zip(bnds[:-1], bnds[1:]):
                rl.append((q, int(c0), int(c1), int(seg_k[c0]),
                           q == 0, bool(stop_f[c0])))
        runs.append(rl)

    # wdiag: col block k (128 wide) holds W[k] replicated on the 4 diagonal
    # 32x32 blocks -> one K=128 matmul computes all 4 lanes at once.
    # Block 8 is zeros for the full-bank start=True PSUM zero-init.
    wrep = np.zeros((128, 9 * 128), np.float32)
    for k in range(8):
        for a in range(4):
            wrep[32 * a:32 * a + 32, 128 * k + 32 * a:128 * k + 32 * a + 32] = W[k]

    in_maps = [{"xin": xin[c].astype(bf16), "wrep": wrep.astype(bf16)}
               for c in range(N_CORES)]

    sched = (CTOT, T, NBLK, tuple(B.tolist()),
             tuple(tuple(row) for row in Poff.tolist()),
             tuple(tuple(rl) for rl in runs))
    dec = dict(vorder=vorder, d_core=v_lane // 4, d_a=v_lane % 4,
               d_blk=v_col // (CHUNK * STAGE),
               d_col=(v_col // CHUNK % STAGE) * CHUNK + v_col % CHUNK,
               num_out=num_out)
    return in_maps, sched, dec


# ----------------------------------------------------------------- program --
def _build_program(sched, dt_name):
    import concourse.tile as tile
    from concourse import bacc, mybir

    CTOT, T, NBLK, B, Poff, runs = sched
    dt = getattr(mybir.dt, dt_name)
    f32 = mybir.dt.float32

    nc = bacc.Bacc("TRN2", target_bir_lowering=False, debug=False)
    xin_d = nc.dram_tensor("xin", [128, CTOT], dt, kind="ExternalInput")
    w_d = nc.dram_tensor("wrep", [128, 9 * 128], dt, kind="ExternalInput")
    out_d = nc.dram_tensor("out_st", [NBLK, 128, CHUNK * STAGE], dt,
                           kind="ExternalOutput")

    with tile.TileContext(nc) as tc:
        with (
            tc.tile_pool(name="w", bufs=1) as wpool,
            tc.tile_pool(name="xb", bufs=1) as xpool,
            tc.tile_pool(name="st", bufs=2) as stpool,
            tc.tile_pool(name="ps", bufs=8, space="PSUM") as pspool,
        ):
            wt = wpool.tile([128, 9 * 128], dt, tag="wt")
            nc.gpsimd.dma_start(wt[:], w_d.ap()[:, :])

            xin = xpool.tile([128, CTOT], dt, tag="xin")
            for t0 in range(0, T, DMA_PAIR):
                c0, c1 = B[t0], B[min(t0 + DMA_PAIR, T)]
                eng = nc.sync if (t0 // DMA_PAIR) % 2 == 0 else nc.scalar
                eng.dma_start(xin[:, c0:c1], xin_d.ap()[:, c0:c1])

            for blk in range(NBLK):
                staging = stpool.tile([128, CHUNK * STAGE], dt, tag="stg")
                for s in range(STAGE):
                    t = blk * STAGE + s
                    ps = pspool.tile([128, CHUNK], f32, tag="ps")
                    # zero-init whole bank + set has_written in ONE instruction
                    # so every data matmul is a pure commutative accumulate
                    nc.tensor.matmul(
                        ps[:, :], wt[:, 8 * 128:9 * 128],
                        wt[:, 0:CHUNK],
                        start=True, stop=False, skip_group_check=True)
                    # k-sorted so consecutive matmuls share stationary weights
                    for (q, c0, c1, k, start, stop) in sorted(
                            runs[t], key=lambda r: (r[3], r[0], r[1])):
                        off = B[t] + Poff[t][q]
                        nc.tensor.matmul(
                            ps[:, c0:c1],
                            wt[:, 128 * k:128 * k + 128],
                            xin[:, off + c0:off + c1],
                            start=False, stop=stop, skip_group_check=True)
                    dst = staging[:, CHUNK * s:CHUNK * (s + 1)]
                    if s % 2 == 0:
                        nc.vector.tensor_copy(dst, ps[:])
                    else:
                        nc.scalar.copy(dst, ps[:])
                nc.gpsimd.dma_start(out_d.ap()[blk], staging[:])

    nc.compile()
    return nc


def _get_program(sched, dt_name):
    key = (hash(sched), dt_name)
    if key not in _prog_cache:
        _prog_cache[key] = _build_program(sched, dt_name)
    return _prog_cache[key]


def _decode(results, dec):
    num_out = dec["num_out"]
    out_st = np.stack([r["out_st"].astype(np.float32) for r in results])
    rows = (32 * dec["d_a"])[:, None] + np.arange(32)[None, :]
    vals = out_st[dec["d_core"][:, None], dec["d_blk"][:, None], rows,
                  dec["d_col"][:, None]]
    res = np.empty((num_out, 32), np.float32)
    res[dec["vorder"]] = vals
    return res


def run(x, W, k_idx, out_idx, num_out, trace=False, dt_name=DT_IN):
    from concourse.bass_utils import run_bass_kernel_spmd

    x = np.asarray(x, dtype=np.float32)
    W = np.asarray(W, dtype=np.float32)
    k_idx = np.asarray(k_idx, dtype=np.int32)
    out_idx = np.asarray(out_idx, dtype=np.int32)
    num_out = int(num_out)

    in_maps, sched, dec = _pack(x, W, k_idx, out_idx, num_out)
    nc = _get_program(sched, dt_name)
    res = run_bass_kernel_spmd(nc, in_maps, list(range(N_CORES)), trace=trace)
    out = _decode(res.results, dec)
    return out, res


def kernel(x, W, k_idx, out_idx, num_out):
    out, _ = run(x, W, k_idx, out_idx, num_out, trace=False)
    return out
